# revision 1
# baseline (speedup 1.0000x reference)
"""Neighbor-slice attention (nn_AttentionModule) on 8 TRN2 NeuronCores.

Layout strategy (per core, 2 of 16 slices + 1 halo slice each side packed by
the host):
  - features slice X[s]: SBUF (128 C-partitions, 2304 hw) padded to 2560
  - qT/kT projections:   (64 ci, hw) via matmul lhsT=[WqT|WkT], rhs=X
  - v projection:        (hw, ci) chunks via lhsT=X_chunk, rhs=WvT, stored
                         bf16 with a ones column appended -> y matmul yields
                         softmax denominators for free (row 64)
  - attention:           fT chunk (128 k, qb) = k_chunk @ qT  (float32r)
                         exp on ACT (or DVE fast-exp) PSUM->SBUF bf16
                         yT_aug (65, qb) accumulated over 18 k-chunks
  - normalize AFTER Wz:  z_unnorm = WzT.T @ y;  out += z_unnorm * bcast(1/d)
  - biases: bq/bk applied on PSUM evac; bv/bz folded into 2*(Wz@bv+bz)
"""

import sys

for _p in ("/opt/trn_rl_repo",):
    if _p not in sys.path:
        sys.path.insert(0, _p)

import numpy as np

N_FULL, C, H, W = 16, 128, 48, 48
HW = H * W            # 2304
CI = C // 2           # 64
QPAD = 2560           # hw padded to a multiple of 512 for fp32r matmul blocks
KC = HW // 128        # 18 k-chunks per slice
NCORES = 8
NLOC = N_FULL // NCORES  # 2 local slices per core

# q-blocks (start, width) in padded coords; width multiple of 512 for fp32r
QBS = [(0, 1024), (1024, 1024), (2048, 512)]

# Schraudolph fast-exp on DVE for a subset of (att, qblock, chunk) units to
# offload the ACT engine.  att in 0..3, qb in 0..2, j in 0..17.
# Filled in by EXP_DVE_PRED; start with everything on ACT.
EXP_DVE_PRED = lambda att, qi, j: False

# bf16 Schraudolph constants: bits16 = round(x*log2(e)*128 + B16)
_S16 = 184.66496736235803          # 2**7 / ln(2)
_B16 = 16256.0 - 4.75              # 127*2**7 with mid-sawtooth correction

_NC_CACHE = {}
LAST_RESULTS = None
TRACE = False


def _build_nc():
    import concourse.bass as bass
    import concourse.mybir as mybir
    import concourse.tile as tile
    from concourse import bacc

    f32 = mybir.dt.float32
    f32r = mybir.dt.float32r
    bf16 = mybir.dt.bfloat16
    i16 = mybir.dt.int16
    FT = mybir.ActivationFunctionType

    nc = bacc.Bacc()

    x4_d = nc.declare_dram_parameter("x4", [4, C, HW], f32, isOutput=False)
    wqk_d = nc.declare_dram_parameter("wqk", [C, C], f32, isOutput=False)
    wv_d = nc.declare_dram_parameter("wv", [C, CI], f32, isOutput=False)
    wz_d = nc.declare_dram_parameter("wz", [CI, C], f32, isOutput=False)
    bqk_d = nc.declare_dram_parameter("bqk", [C, 1], f32, isOutput=False)
    c2_d = nc.declare_dram_parameter("c2", [C, 1], f32, isOutput=False)
    out_d = nc.declare_dram_parameter("out", [NLOC, C, HW], f32, isOutput=True)
    dbg_qt_d = nc.declare_dram_parameter("dbg_qt", [CI, QPAD], f32, isOutput=True)
    dbg_kt_d = nc.declare_dram_parameter("dbg_kt", [CI, HW], f32, isOutput=True)
    dbg_at_d = nc.declare_dram_parameter("dbg_at", [C, 1024], f32, isOutput=True)
    dbg_y_d = nc.declare_dram_parameter("dbg_y", [CI + 1, 1024], f32, isOutput=True)
    dbg_rb_d = nc.declare_dram_parameter("dbg_rb", [1, 1024], f32, isOutput=True)
    dbg_rbc_d = nc.declare_dram_parameter("dbg_rbc", [C, 1024], f32, isOutput=True)

    def r32(ap):
        return ap.bitcast(f32r)

    with tile.TileContext(nc) as tc:
        with tc.tile_pool(name="const", bufs=1) as cpool, \
             tc.tile_pool(name="xt", bufs=4) as xpool, \
             tc.tile_pool(name="qt", bufs=2) as qtpool, \
             tc.tile_pool(name="kt", bufs=4) as ktpool, \
             tc.tile_pool(name="vg", bufs=4) as vgpool, \
             tc.tile_pool(name="at", bufs=3) as atpool, \
             tc.tile_pool(name="ysb", bufs=2) as ypool, \
             tc.tile_pool(name="rb", bufs=2) as rbpool, \
             tc.tile_pool(name="u0", bufs=4) as u0pool, \
             tc.tile_pool(name="u1", bufs=2) as u1pool, \
             tc.tile_pool(name="osb", bufs=3) as opool:

            # ---- constants ----
            wqk_t = cpool.tile([C, C], f32, tag="wqk")
            wv_t = cpool.tile([C, CI], f32, tag="wv")
            wzf_t = cpool.tile([CI, C], f32, tag="wzf")
            wz_t = cpool.tile([CI, C], bf16, tag="wz")
            bqk_t = cpool.tile([C, 1], f32, tag="bqk")
            c2_t = cpool.tile([C, 1], f32, tag="c2")

            nc.sync.dma_start(out=wqk_t, in_=wqk_d[:, :])
            nc.sync.dma_start(out=wv_t, in_=wv_d[:, :])
            nc.sync.dma_start(out=wzf_t, in_=wz_d[:, :])
            nc.sync.dma_start(out=bqk_t, in_=bqk_d[:, :])
            nc.sync.dma_start(out=c2_t, in_=c2_d[:, :])
            nc.vector.tensor_copy(wz_t, wzf_t)


            # ---- load features, pad to QPAD ----
            x_t = []
            for s in range(4):
                xt = xpool.tile([C, QPAD], f32, tag="xt")
                nc.sync.dma_start(out=xt[:, 0:HW], in_=x4_d[s])
                nc.gpsimd.memset(xt[:, HW:QPAD], 0.0)
                x_t.append(xt)

            # ---- projections ----
            qt_t = [None, None]      # local slices only (x4 idx 1, 2)
            kt_t = [None] * 4
            vg_t = [None] * 4
            with tc.tile_pool(name="pp", bufs=1, space="PSUM") as pp, \
                 tc.tile_pool(name="pv", bufs=1, space="PSUM") as pv:
                for s in range(4):
                    pq = pp.tile([C, QPAD], f32, tag="pp")
                    for b in range(QPAD // 512):
                        sl = slice(512 * b, 512 * (b + 1))
                        nc.tensor.matmul(pq[:, sl], lhsT=wqk_t,
                                         rhs=x_t[s][:, sl],
                                         start=True, stop=True)
                    if s in (1, 2):
                        qt = qtpool.tile([CI, QPAD], f32r, tag="qt")
                        nc.scalar.activation(qt, pq[0:CI, :], FT.Identity,
                                             bias=bqk_t[0:CI, :])
                        qt_t[s - 1] = qt
                    kt = ktpool.tile([CI, HW], f32r, tag="kt")
                    nc.vector.tensor_scalar_add(kt, pq[CI:C, 0:HW],
                                                bqk_t[CI:C, :])
                    kt_t[s] = kt
                    if s == 0:
                        nc.sync.dma_start(out=dbg_kt_d[:, :], in_=kt.bitcast(f32))
                    if s == 1:
                        nc.sync.dma_start(out=dbg_qt_d[:, :],
                                          in_=qt_t[0].bitcast(f32))

                    pvt = pv.tile([C, KC * CI], f32, tag="pv")
                    for j in range(KC):
                        nc.tensor.matmul(pvt[:, CI * j:CI * (j + 1)],
                                         lhsT=x_t[s][:, 128 * j:128 * (j + 1)],
                                         rhs=wv_t, start=True, stop=True)
                    vg = vgpool.tile([C, KC, CI + 1], bf16, tag="vg")
                    nc.scalar.activation(
                        vg[:, :, 0:CI],
                        pvt.rearrange("p (j d) -> p j d", d=CI), FT.Copy)
                    nc.gpsimd.memset(vg[:, :, CI], 1.0)
                    vg_t[s] = vg

            # ---- attention ----
            with tc.tile_pool(name="pf", bufs=2, space="PSUM") as pf, \
                 tc.tile_pool(name="pacc", bufs=2, space="PSUM") as pacc:
                att = 0
                for n in range(NLOC):
                    ub = [None, None, None]
                    for side in range(2):          # 0: before, 1: after
                        kv = n + (0 if side == 0 else 2)
                        for qi, (q0, w) in enumerate(QBS):
                            real = min(w, HW - q0)
                            yps = pacc.tile([CI + 1, w], f32, tag="acc")
                            for j in range(KC):
                                ft = pf.tile([C, w], f32, tag="ft")
                                for b in range(w // 512):
                                    sl = slice(512 * b, 512 * (b + 1))
                                    qsl = slice(q0 + 512 * b, q0 + 512 * (b + 1))
                                    nc.tensor.matmul(
                                        ft[:, sl],
                                        lhsT=kt_t[kv][:, 128 * j:128 * (j + 1)],
                                        rhs=qt_t[n][:, qsl],
                                        start=True, stop=True)
                                at = atpool.tile([C, w], bf16, tag="at")
                                if EXP_DVE_PRED(att, qi, j):
                                    nc.vector.tensor_scalar(
                                        at.bitcast(i16)[:, 0:real],
                                        ft[:, 0:real], _S16, _B16,
                                        op0=mybir.AluOpType.mult,
                                        op1=mybir.AluOpType.add)
                                else:
                                    nc.scalar.activation(at[:, 0:real],
                                                         ft[:, 0:real], FT.Exp)
                                for b in range(0, real, 512):
                                    e = min(b + 512, real)
                                    nc.tensor.matmul(yps[:, b:e],
                                                     lhsT=vg_t[kv][:, j, :],
                                                     rhs=at[:, b:e],
                                                     start=(j == 0),
                                                     stop=(j == KC - 1))
                            if att == 0 and qi == 0:
                                dbg_y = opool.tile([CI + 1, w], f32, tag="osb")
                                nc.scalar.activation(dbg_y, yps, FT.Identity)
                                nc.sync.dma_start(out=dbg_y_d[:, :], in_=dbg_y)
                            rbt = rbpool.tile([1, w], f32, tag="rb")
                            nc.vector.reciprocal(rbt[:, 0:real],
                                                 yps[CI:CI + 1, 0:real])
                            if att == 0 and qi == 0:
                                nc.sync.dma_start(out=dbg_rb_d[:, :],
                                                  in_=rbt)
                            ysb = ypool.tile([CI, w], bf16, tag="ysb")
                            nc.scalar.activation(ysb[:, 0:real],
                                                 yps[0:CI, 0:real], FT.Copy)
                            zps = pacc.tile([C, w], f32, tag="acc")
                            for b in range(0, real, 512):
                                e = min(b + 512, real)
                                nc.tensor.matmul(zps[:, b:e], lhsT=wz_t,
                                                 rhs=ysb[:, b:e],
                                                 start=True, stop=True)
                            rbc = rbpool.tile([C, w], f32, tag="rbc")
                            nc.gpsimd.partition_broadcast(rbc[:, 0:real],
                                                          rbt[:, 0:real])
                            if att == 0 and qi == 0:
                                nc.sync.dma_start(out=dbg_rbc_d[:, :], in_=rbc)
                            if side == 0:
                                u = u0pool.tile([C, w], f32, tag="u0")
                                nc.vector.tensor_mul(u[:, 0:real],
                                                      zps[:, 0:real],
                                                      rbc[:, 0:real])
                                ub[qi] = u
                            else:
                                ua = u1pool.tile([C, w], f32, tag="u1")
                                nc.vector.tensor_mul(ua[:, 0:real],
                                                      zps[:, 0:real],
                                                      rbc[:, 0:real])
                                vv = u1pool.tile([C, w], f32, tag="u1")
                                nc.vector.scalar_tensor_tensor(
                                    out=vv[:, 0:real], in0=ub[qi][:, 0:real],
                                    scalar=c2_t, in1=ua[:, 0:real],
                                    op0=mybir.AluOpType.add,
                                    op1=mybir.AluOpType.add)
                                osb = opool.tile([C, w], f32, tag="osb")
                                nc.vector.tensor_add(
                                    osb[:, 0:real], vv[:, 0:real],
                                    x_t[n + 1][:, q0:q0 + real])
                                nc.sync.dma_start(
                                    out=out_d[n][:, q0:q0 + real],
                                    in_=osb[:, 0:real])
                        att += 1

    nc.compile()
    return nc


def _get_nc():
    if "nc" not in _NC_CACHE:
        _NC_CACHE["nc"] = _build_nc()
    return _NC_CACHE["nc"]


def _host_inputs(features, Wq, bq, Wk, bk, Wv, bv, Wz, bz):
    X = np.ascontiguousarray(np.asarray(features, np.float32).reshape(N_FULL, C, HW))
    wqk = np.ascontiguousarray(np.concatenate([Wq.T, Wk.T], axis=1), np.float32)
    wv = np.ascontiguousarray(np.asarray(Wv).T, np.float32)
    wz = np.ascontiguousarray(np.asarray(Wz).T, np.float32)
    bqk = np.concatenate([bq, bk]).astype(np.float32).reshape(C, 1)
    c2 = (2.0 * (np.asarray(Wz) @ np.asarray(bv) + np.asarray(bz))).astype(
        np.float32).reshape(C, 1)
    in_maps = []
    for i in range(NCORES):
        idx = [max(2 * i - 1, 0), 2 * i, 2 * i + 1, min(2 * i + 2, N_FULL - 1)]
        in_maps.append({
            "x4": np.ascontiguousarray(X[idx]),
            "wqk": wqk, "wv": wv, "wz": wz, "bqk": bqk, "c2": c2,
        })
    return in_maps


def kernel(features, Wq, bq, Wk, bk, Wv, bv, Wz, bz):
    global LAST_RESULTS
    from concourse.bass_utils import run_bass_kernel_spmd

    nc = _get_nc()
    in_maps = _host_inputs(features, Wq, bq, Wk, bk, Wv, bv, Wz, bz)
    res = run_bass_kernel_spmd(nc, in_maps, core_ids=list(range(NCORES)),
                               trace=TRACE)
    LAST_RESULTS = res
    out = np.empty((N_FULL, C, H, W), np.float32)
    for i in range(NCORES):
        out[2 * i:2 * i + 2] = res.results[i]["out"].reshape(NLOC, C, H, W)
    return out

